# revision 25
# baseline (speedup 1.0000x reference)
"""Trainium2 Bass kernel for nn_Attention_84585085927925 — bf16 M-folded Gram.

Reference (per batch element b, all fp32):
    qkv = x @ w_qkv.T ; q,k,v heads of 64 ; attn = sqrt(64) * q @ k.T (NO
    softmax) ; out = attn @ v ; out = out @ w_fc.T + b_fc

No softmax => attention is linear; fold k/v AND the q/fc projections into a
single per-batch effective matrix M:
    out = x @ M + b_fc,   M = s * sum_h wq_h.T G_h wfc[:,h].T,
    G_h = wk_h C wv_h.T,  C = x.T x  (symmetric)
Per-core pipeline (one batch element per NeuronCore, 8 cores):
    C    upper-triangle blocks via PSUM, mirrored by PE transpose
    T1   = C @ wv.T                       [768,768]
    G    = wk_pair @ T1[:,pair]  (block-diag per head pair)
    M1T  = G_blkdiag.T @ (s*wq)_pair      [128,768] per pair
    M    = M1T.T @ wfc.T                  [768,768]
    outT = M.T @ xT + b_fc                [768,1024]
All matmuls bf16 (1 cyc/row, FWL weight loads); fp32 PSUM accumulate.
"""

import numpy as np
import ml_dtypes

import concourse.bass as bass  # noqa: F401  (registers engine namespaces)
import concourse.mybir as mybir
import concourse.tile as tile
from concourse import bacc, bass_utils

F32 = mybir.dt.float32
BF16 = mybir.dt.bfloat16
U32 = mybir.dt.uint32

B, N, D, H = 8, 1024, 768, 12
HD = D // H            # 64
SCALE = float(np.sqrt(HD))
DT = D // 128           # 6 blocks of 128 along feature dims
NT = N // 128           # 8 token tiles


def _build_program(debug_dumps=False, num_devices=B):
    nc = bacc.Bacc(
        trn_type="TRN2", target_bir_lowering=False, debug=False,
        num_devices=num_devices
    )
    xN_d = nc.dram_tensor("xN", [N, D], BF16, kind="ExternalInput").ap()
    xT_d = nc.dram_tensor("xT", [D, N], BF16, kind="ExternalInput").ap()
    wvT_d = nc.dram_tensor("wvT", [D, D], BF16, kind="ExternalInput").ap()
    wkT_d = nc.dram_tensor("wkT", [D, D], BF16, kind="ExternalInput").ap()
    wq_d = nc.dram_tensor("wq", [D, D], BF16, kind="ExternalInput").ap()
    wfcT_d = nc.dram_tensor("wfcT", [D, D], BF16, kind="ExternalInput").ap()
    bfc_d = nc.dram_tensor("bfc", [D], F32, kind="ExternalInput").ap()
    id_d = nc.dram_tensor("ident", [128, 128], BF16, kind="ExternalInput").ap()
    outT_d = nc.dram_tensor("outT", [D, N], BF16, kind="ExternalOutput").ap()
    dbg = {}
    if debug_dumps:
        for nm, shape in [("d_c", [128, DT, D]), ("d_t1", [128, DT, D]),
                          ("d_g2", [128, DT, 128]), ("d_m1t", [128, DT, D]),
                          ("d_m", [128, DT, D])]:
            dbg[nm] = nc.dram_tensor(nm, shape, BF16,
                                     kind="ExternalOutput").ap()

    with tile.TileContext(nc) as tc:
        with tc.tile_pool(name="big", bufs=1) as big, \
             tc.tile_pool(name="outsp", bufs=3) as outsp, \
             tc.tile_pool(name="psa", bufs=4, space="PSUM") as psa, \
             tc.tile_pool(name="psw", bufs=2, space="PSUM") as psw, \
             tc.tile_pool(name="psg", bufs=2, space="PSUM") as psg:

        # PSUM static budget: psa 4 banks + psw 2 + psg 2 = 8.

            xn_t = [big.tile([128, D], BF16, name=f"xn{o}") for o in range(NT)]
            xT_sb = big.tile([128, DT, N], BF16, name="xT_sb")
            wvT_sb = big.tile([128, DT, D], BF16, name="wvT_sb")
            wkT_sb = big.tile([128, DT, D], BF16, name="wkT_sb")
            wq_sb = big.tile([128, DT, D], BF16, name="wq_sb")
            wfcT_sb = big.tile([128, DT, D], BF16, name="wfcT_sb")
            c_sb = big.tile([128, DT, D], BF16, name="c_sb")
            t1_sb = big.tile([128, DT, D], BF16, name="t1_sb")
            g2_sb = big.tile([128, DT, 128], BF16, name="g2_sb")
            m1t_sb = big.tile([128, DT, D], BF16, name="m1t_sb")
            m_sb = big.tile([128, DT, D], BF16, name="m_sb")
            bias_sb = big.tile([128, DT], F32, name="bias_sb")
            id_sb = big.tile([128, 128], BF16, name="id_sb")
            scr_sb = big.tile([128, 512], BF16, name="scr_sb")

            xN_r = xN_d.rearrange("(o p) e -> p o e", p=128)
            xT_r = xT_d.rearrange("(o p) n -> p o n", p=128)
            wvT_r = wvT_d.rearrange("(o p) e -> p o e", p=128)
            wkT_r = wkT_d.rearrange("(o p) e -> p o e", p=128)
            wq_r = wq_d.rearrange("(o p) e -> p o e", p=128)
            wfcT_r = wfcT_d.rearrange("(o p) e -> p o e", p=128)
            outT_r = outT_d.rearrange("(o p) n -> p o n", p=128)

            # ---- DMA in on two hwdge queues (sync + scalar) ----
            # sync queue: xN even tiles first (C starts immediately)
            # scalar queue: xN odd tiles, then remaining weights
            nc.sync.dma_start(xn_t[0][:], xN_r[:, 0, :])
            nc.scalar.dma_start(xn_t[1][:], xN_r[:, 1, :])
            nc.sync.dma_start(xn_t[2][:], xN_r[:, 2, :])
            nc.scalar.dma_start(xn_t[3][:], xN_r[:, 3, :])
            nc.sync.dma_start(xn_t[4][:], xN_r[:, 4, :])
            nc.scalar.dma_start(xn_t[5][:], xN_r[:, 5, :])
            nc.sync.dma_start(xn_t[6][:], xN_r[:, 6, :])
            nc.scalar.dma_start(xn_t[7][:], xN_r[:, 7, :])
            nc.sync.dma_start(id_sb[:], id_d)
            nc.sync.dma_start(bias_sb[:], bfc_d.rearrange("(o p) -> p o", p=128))
            # strict first-needed priority, halves split across both queues
            s0, s1 = slice(0, 3), slice(3, 6)
            nc.sync.dma_start(wvT_sb[:, s0, :], wvT_r[:, s0, :])
            nc.scalar.dma_start(wvT_sb[:, s1, :], wvT_r[:, s1, :])
            nc.sync.dma_start(wkT_sb[:, s0, :], wkT_r[:, s0, :])
            nc.scalar.dma_start(wkT_sb[:, s1, :], wkT_r[:, s1, :])
            nc.sync.dma_start(wq_sb[:, s0, :], wq_r[:, s0, :])
            nc.scalar.dma_start(wq_sb[:, s1, :], wq_r[:, s1, :])
            nc.sync.dma_start(wfcT_sb[:, s0, :], wfcT_r[:, s0, :])
            nc.scalar.dma_start(wfcT_sb[:, s1, :], wfcT_r[:, s1, :])
            nc.sync.dma_start(xT_sb[:, s0, :], xT_r[:, s0, :])
            nc.scalar.dma_start(xT_sb[:, s1, :], xT_r[:, s1, :])

            # zero g2 once (gpsimd — off critical path)
            nc.gpsimd.memset(g2_sb[:], 0.0)

            # warm up the PE p-state while the first xN DMA is in flight
            nc.vector.memset(scr_sb[:], 0.0)
            wu = psw.tile([128, 512], F32, tag="w", name="wu")
            for k in range(10):
                nc.tensor.matmul(wu[:, :512], scr_sb[:, 0:128],
                                 scr_sb[:, :512], start=(k == 0),
                                 stop=(k == 9))

            copy_engines = [nc.vector.tensor_copy, nc.scalar.copy]
            ce_idx = [0]

            def copy(dst, src):
                copy_engines[ce_idx[0] % 2](dst, src)
                ce_idx[0] += 1

            # ---- C = x.T x, upper triangle, 3 phases of 2 rows ----
            # row r covers cols r*128:768 (split into <=512 chunks)
            row_chunks = {0: [(0, 512), (512, 256)], 1: [(128, 512), (640, 128)],
                          2: [(256, 512)], 3: [(384, 384)],
                          4: [(512, 256)], 5: [(640, 128)]}

            def c_phase(rows):
                tiles = []
                for r in rows:
                    for c0, w in row_chunks[r]:
                        pt = psa.tile([128, 512], F32, tag="a", name=f"c{r}_{c0}")
                        tiles.append((r, c0, w, pt))
                for nt in range(NT):
                    for r, c0, w, pt in tiles:
                        nc.tensor.matmul(
                            pt[:, :w],
                            xn_t[nt][:, r * 128:(r + 1) * 128],
                            xn_t[nt][:, c0:c0 + w],
                            start=(nt == 0), stop=(nt == NT - 1),
                        )
                for r, c0, w, pt in tiles:
                    copy(c_sb[:, r, c0:c0 + w], pt[:, :w])

            def emit_mirror(i, j):
                # slot (j, i) := transpose of stored upper block (i, j)
                tp = psg.tile([128, 128], BF16, tag="g", name=f"tr{i}{j}")
                nc.tensor.transpose(
                    tp[:], c_sb[:, i, j * 128:(j + 1) * 128], id_sb[:]
                )
                nc.vector.tensor_copy(
                    c_sb[:, j, i * 128:(i + 1) * 128].bitcast(U32),
                    tp[:].bitcast(U32))

            def t1_stage(i):
                ua = psa.tile([128, 512], F32, tag="a", name=f"t1a{i}")
                ub = psa.tile([128, 512], F32, tag="a", name=f"t1b{i}")
                for j in range(DT):
                    lhsT = c_sb[:, j, i * 128:(i + 1) * 128]
                    nc.tensor.matmul(ua[:, :512], lhsT, wvT_sb[:, j, 0:512],
                                     start=(j == 0), stop=(j == DT - 1))
                    nc.tensor.matmul(ub[:, :256], lhsT, wvT_sb[:, j, 512:768],
                                     start=(j == 0), stop=(j == DT - 1))
                copy(t1_sb[:, i, 0:512], ua[:, :512])
                copy(t1_sb[:, i, 512:768], ub[:, :256])

            c_phase([0, 1])
            c_phase([2, 3])
            for i, j in [(0, 1), (0, 2), (0, 3), (0, 4), (0, 5),
                         (1, 2), (1, 3), (1, 4), (1, 5)]:
                emit_mirror(i, j)
            # T1 i=0,1 depend only on rows 0/1 blocks + batch-1 mirrors —
            # run them before C phase 4/5 to cover the mirror-copy latency
            t1_stage(0)
            c_phase([4, 5])
            t1_stage(1)
            for i, j in [(2, 3), (2, 4), (2, 5), (3, 4), (3, 5), (4, 5)]:
                emit_mirror(i, j)
            for i in range(2, DT):
                t1_stage(i)

            # ---- G (block-diag per pair) + M1T + M, software-pipelined ----
            def g_stage(t):
                gp = psg.tile([128, 128], F32, tag="g", name=f"g{t}")
                for j in range(DT):
                    nc.tensor.matmul(
                        gp[:],
                        wkT_sb[:, j, t * 128:(t + 1) * 128],
                        t1_sb[:, j, t * 128:(t + 1) * 128],
                        start=(j == 0), stop=(j == DT - 1),
                    )
                copy(g2_sb[0:64, t, 0:64], gp[0:64, 0:64])
                copy(g2_sb[64:128, t, 64:128], gp[64:128, 64:128])

            def m1t_stage(t):
                pa = psw.tile([128, 512], F32, tag="w", name=f"m1a{t}")
                pb = psw.tile([128, 512], F32, tag="w", name=f"m1b{t}")
                nc.tensor.matmul(pa[:, :512], g2_sb[:, t, :],
                                 wq_sb[:, t, 0:512], start=True, stop=True)
                nc.tensor.matmul(pb[:, :256], g2_sb[:, t, :],
                                 wq_sb[:, t, 512:768], start=True, stop=True)
                copy(m1t_sb[:, t, 0:512], pa[:, :512])
                copy(m1t_sb[:, t, 512:768], pb[:, :256])

            # M accumulators for dblocks 0,1 live across the G/M1T pipeline
            mtiles01 = []
            for db in (0, 1):
                ma = psa.tile([128, 512], F32, tag="a", name=f"ma{db}")
                mb = psa.tile([128, 512], F32, tag="a", name=f"mb{db}")
                mtiles01.append((db, ma, mb))

            def m_step(dbtiles, t):
                for db, ma, mb in dbtiles:
                    lhsT = m1t_sb[:, t, db * 128:(db + 1) * 128]
                    nc.tensor.matmul(ma[:, :512], lhsT, wfcT_sb[:, t, 0:512],
                                     start=(t == 0), stop=(t == DT - 1))
                    nc.tensor.matmul(mb[:, :256], lhsT, wfcT_sb[:, t, 512:768],
                                     start=(t == 0), stop=(t == DT - 1))

            g_stage(0)
            g_stage(1)
            m1t_stage(0)
            g_stage(2)
            m1t_stage(1)
            m_step(mtiles01, 0)
            g_stage(3)
            m1t_stage(2)
            m_step(mtiles01, 1)
            g_stage(4)
            m1t_stage(3)
            m_step(mtiles01, 2)
            g_stage(5)
            m1t_stage(4)
            m_step(mtiles01, 3)
            m1t_stage(5)
            m_step(mtiles01, 4)
            m_step(mtiles01, 5)
            for db, ma, mb in mtiles01:
                copy(m_sb[:, db, 0:512], ma[:, :512])
                copy(m_sb[:, db, 512:768], mb[:, :256])
            for phase in (1, 2):
                mts = []
                for db in (phase * 2, phase * 2 + 1):
                    ma = psa.tile([128, 512], F32, tag="a", name=f"ma{db}")
                    mb = psa.tile([128, 512], F32, tag="a", name=f"mb{db}")
                    mts.append((db, ma, mb))
                for t in range(DT):
                    m_step(mts, t)
                for db, ma, mb in mts:
                    copy(m_sb[:, db, 0:512], ma[:, :512])
                    copy(m_sb[:, db, 512:768], mb[:, :256])

            # ---- outT = M.T @ xT + bias ----
            for et in range(DT):
                oa = psa.tile([128, 512], F32, tag="a", name=f"oa{et}")
                ob = psa.tile([128, 512], F32, tag="a", name=f"ob{et}")
                ot = outsp.tile([128, N], BF16, tag="ot", name=f"ot{et}")
                if et < DT - 1:
                    for dt in range(DT):
                        lhsT = m_sb[:, dt, et * 128:(et + 1) * 128]
                        nc.tensor.matmul(oa[:, :512], lhsT,
                                         xT_sb[:, dt, 0:512],
                                         start=(dt == 0), stop=(dt == DT - 1))
                        nc.tensor.matmul(ob[:, :512], lhsT,
                                         xT_sb[:, dt, 512:1024],
                                         start=(dt == 0), stop=(dt == DT - 1))
                    nc.scalar.add(ot[:, 0:512], oa[:, :512],
                                  bias_sb[:, et:et + 1])
                    nc.vector.tensor_scalar_add(ot[:, 512:1024], ob[:, :512],
                                                bias_sb[:, et:et + 1])
                    nc.sync.dma_start(outT_r[:, et, :], ot[:])
                else:
                    # last tile: finish the low n-half first so its bias+DMA
                    # overlaps the high half's matmuls, shortening the tail
                    for dt in range(DT):
                        nc.tensor.matmul(oa[:, :512],
                                         m_sb[:, dt, et * 128:(et + 1) * 128],
                                         xT_sb[:, dt, 0:512],
                                         start=(dt == 0), stop=(dt == DT - 1))
                    nc.scalar.add(ot[:, 0:256], oa[:, 0:256],
                                  bias_sb[:, et:et + 1])
                    nc.sync.dma_start(outT_r[:, et, 0:256], ot[:, 0:256])
                    nc.vector.tensor_scalar_add(ot[:, 256:512], oa[:, 256:512],
                                                bias_sb[:, et:et + 1])
                    nc.scalar.dma_start(outT_r[:, et, 256:512], ot[:, 256:512])
                    for dt in range(DT):
                        nc.tensor.matmul(ob[:, :512],
                                         m_sb[:, dt, et * 128:(et + 1) * 128],
                                         xT_sb[:, dt, 512:1024],
                                         start=(dt == 0), stop=(dt == DT - 1))
                    nc.scalar.add(ot[:, 512:768], ob[:, 0:256],
                                  bias_sb[:, et:et + 1])
                    nc.sync.dma_start(outT_r[:, et, 512:768], ot[:, 512:768])
                    nc.vector.tensor_scalar_add(ot[:, 768:1024],
                                                ob[:, 256:512],
                                                bias_sb[:, et:et + 1])
                    nc.scalar.dma_start(outT_r[:, et, 768:1024],
                                        ot[:, 768:1024])

            if debug_dumps:
                for nm, sb in [("d_c", c_sb), ("d_t1", t1_sb),
                               ("d_g2", g2_sb), ("d_m1t", m1t_sb),
                               ("d_m", m_sb)]:
                    nc.sync.dma_start(dbg[nm], sb)

    nc.compile()
    return nc


_NC_CACHE = None
LAST_EXEC_NS = None
LAST_RES = None


def kernel(x, w_qkv, w_fc, b_fc, _trace=False):
    global _NC_CACHE, LAST_EXEC_NS, LAST_RES
    x = np.asarray(x, dtype=np.float32)
    w_qkv = np.asarray(w_qkv, dtype=np.float32)
    w_fc = np.asarray(w_fc, dtype=np.float32)
    b_fc = np.asarray(b_fc, dtype=np.float32)

    if _NC_CACHE is None:
        _NC_CACHE = _build_program()
    nc = _NC_CACHE

    bf = ml_dtypes.bfloat16
    wvT = np.ascontiguousarray(w_qkv[2 * D:].T).astype(bf)
    wkT = np.ascontiguousarray(w_qkv[D:2 * D].T).astype(bf)
    wq = np.ascontiguousarray(SCALE * w_qkv[:D]).astype(bf)
    wfcT = np.ascontiguousarray(w_fc.T).astype(bf)
    ident = np.eye(128, dtype=bf)

    in_maps = []
    for b in range(B):
        in_maps.append({
            "xN": x[b].astype(bf),
            "xT": np.ascontiguousarray(x[b].T).astype(bf),
            "wvT": wvT, "wkT": wkT, "wq": wq, "wfcT": wfcT,
            "bfc": b_fc, "ident": ident,
        })

    res = bass_utils.run_bass_kernel_spmd(
        nc, in_maps, core_ids=list(range(B)), trace=_trace
    )
    LAST_EXEC_NS = res.exec_time_ns
    LAST_RES = res
    out = np.stack([res.results[b]["outT"].astype(np.float32).T
                    for b in range(B)])
    return np.ascontiguousarray(out)


# revision 27
# speedup vs baseline: 1.0029x; 1.0029x over previous
"""Trainium2 Bass kernel for nn_Attention_84585085927925 — bf16 M-folded Gram.

Reference (per batch element b, all fp32):
    qkv = x @ w_qkv.T ; q,k,v heads of 64 ; attn = sqrt(64) * q @ k.T (NO
    softmax) ; out = attn @ v ; out = out @ w_fc.T + b_fc

No softmax => attention is linear; fold k/v AND the q/fc projections into a
single per-batch effective matrix M:
    out = x @ M + b_fc,   M = s * sum_h wq_h.T G_h wfc[:,h].T,
    G_h = wk_h C wv_h.T,  C = x.T x  (symmetric)
Per-core pipeline (one batch element per NeuronCore, 8 cores):
    C    upper-triangle blocks via PSUM, mirrored by PE transpose
    T1   = C @ wv.T                       [768,768]
    G    = wk_pair @ T1[:,pair]  (block-diag per head pair)
    M1T  = G_blkdiag.T @ (s*wq)_pair      [128,768] per pair
    M    = M1T.T @ wfc.T                  [768,768]
    outT = M.T @ xT + b_fc                [768,1024]
All matmuls bf16 (1 cyc/row, FWL weight loads); fp32 PSUM accumulate.
"""

import numpy as np
import ml_dtypes

import concourse.bass as bass  # noqa: F401  (registers engine namespaces)
import concourse.mybir as mybir
import concourse.tile as tile
from concourse import bacc, bass_utils

F32 = mybir.dt.float32
BF16 = mybir.dt.bfloat16
U32 = mybir.dt.uint32

B, N, D, H = 8, 1024, 768, 12
HD = D // H            # 64
SCALE = float(np.sqrt(HD))
DT = D // 128           # 6 blocks of 128 along feature dims
NT = N // 128           # 8 token tiles


def _build_program(debug_dumps=False, num_devices=B):
    nc = bacc.Bacc(
        trn_type="TRN2", target_bir_lowering=False, debug=False,
        num_devices=num_devices
    )
    xN_d = nc.dram_tensor("xN", [N, D], BF16, kind="ExternalInput").ap()
    xT_d = nc.dram_tensor("xT", [D, N], BF16, kind="ExternalInput").ap()
    wvT_d = nc.dram_tensor("wvT", [D, D], BF16, kind="ExternalInput").ap()
    wkT_d = nc.dram_tensor("wkT", [D, D], BF16, kind="ExternalInput").ap()
    wq_d = nc.dram_tensor("wq", [D, D], BF16, kind="ExternalInput").ap()
    wfcT_d = nc.dram_tensor("wfcT", [D, D], BF16, kind="ExternalInput").ap()
    bfc_d = nc.dram_tensor("bfc", [D], F32, kind="ExternalInput").ap()
    id_d = nc.dram_tensor("ident", [128, 128], BF16, kind="ExternalInput").ap()
    outT_d = nc.dram_tensor("outT", [D, N], BF16, kind="ExternalOutput").ap()
    dbg = {}
    if debug_dumps:
        for nm, shape in [("d_c", [128, DT, D]), ("d_t1", [128, DT, D]),
                          ("d_g2", [128, DT, 128]), ("d_m1t", [128, DT, D]),
                          ("d_m", [128, DT, D])]:
            dbg[nm] = nc.dram_tensor(nm, shape, BF16,
                                     kind="ExternalOutput").ap()

    with tile.TileContext(nc) as tc:
        with tc.tile_pool(name="big", bufs=1) as big, \
             tc.tile_pool(name="outsp", bufs=3) as outsp, \
             tc.tile_pool(name="psa", bufs=4, space="PSUM") as psa, \
             tc.tile_pool(name="psw", bufs=2, space="PSUM") as psw, \
             tc.tile_pool(name="psg", bufs=2, space="PSUM") as psg:

        # PSUM static budget: psa 4 banks + psw 2 + psg 2 = 8.

            xn_t = [big.tile([128, D], BF16, name=f"xn{o}") for o in range(NT)]
            xT_sb = big.tile([128, DT, N], BF16, name="xT_sb")
            wvT_sb = big.tile([128, DT, D], BF16, name="wvT_sb")
            wkT_sb = big.tile([128, DT, D], BF16, name="wkT_sb")
            wq_sb = big.tile([128, DT, D], BF16, name="wq_sb")
            wfcT_sb = big.tile([128, DT, D], BF16, name="wfcT_sb")
            c_sb = big.tile([128, DT, D], BF16, name="c_sb")
            t1_sb = big.tile([128, DT, D], BF16, name="t1_sb")
            g2_sb = big.tile([128, DT, 128], BF16, name="g2_sb")
            m1t_sb = big.tile([128, DT, D], BF16, name="m1t_sb")
            m_sb = big.tile([128, DT, D], BF16, name="m_sb")
            bias_sb = big.tile([128, DT], F32, name="bias_sb")
            id_sb = big.tile([128, 128], BF16, name="id_sb")
            scr_sb = big.tile([128, 512], BF16, name="scr_sb")

            xN_r = xN_d.rearrange("(o p) e -> p o e", p=128)
            xT_r = xT_d.rearrange("(o p) n -> p o n", p=128)
            wvT_r = wvT_d.rearrange("(o p) e -> p o e", p=128)
            wkT_r = wkT_d.rearrange("(o p) e -> p o e", p=128)
            wq_r = wq_d.rearrange("(o p) e -> p o e", p=128)
            wfcT_r = wfcT_d.rearrange("(o p) e -> p o e", p=128)
            outT_r = outT_d.rearrange("(o p) n -> p o n", p=128)

            # ---- DMA in on two hwdge queues (sync + scalar) ----
            # sync queue: xN even tiles first (C starts immediately)
            # scalar queue: xN odd tiles, then remaining weights
            nc.sync.dma_start(xn_t[0][:], xN_r[:, 0, :])
            nc.scalar.dma_start(xn_t[1][:], xN_r[:, 1, :])
            nc.sync.dma_start(xn_t[2][:], xN_r[:, 2, :])
            nc.scalar.dma_start(xn_t[3][:], xN_r[:, 3, :])
            nc.sync.dma_start(xn_t[4][:], xN_r[:, 4, :])
            nc.scalar.dma_start(xn_t[5][:], xN_r[:, 5, :])
            nc.sync.dma_start(xn_t[6][:], xN_r[:, 6, :])
            nc.scalar.dma_start(xn_t[7][:], xN_r[:, 7, :])
            nc.sync.dma_start(id_sb[:], id_d)
            nc.sync.dma_start(bias_sb[:], bfc_d.rearrange("(o p) -> p o", p=128))
            # strict first-needed priority, halves split across both queues
            s0, s1 = slice(0, 3), slice(3, 6)
            nc.sync.dma_start(wvT_sb[:, s0, :], wvT_r[:, s0, :])
            nc.scalar.dma_start(wvT_sb[:, s1, :], wvT_r[:, s1, :])
            nc.sync.dma_start(wkT_sb[:, s0, :], wkT_r[:, s0, :])
            nc.scalar.dma_start(wkT_sb[:, s1, :], wkT_r[:, s1, :])
            nc.sync.dma_start(wq_sb[:, s0, :], wq_r[:, s0, :])
            nc.scalar.dma_start(wq_sb[:, s1, :], wq_r[:, s1, :])
            nc.sync.dma_start(wfcT_sb[:, s0, :], wfcT_r[:, s0, :])
            nc.scalar.dma_start(wfcT_sb[:, s1, :], wfcT_r[:, s1, :])
            nc.sync.dma_start(xT_sb[:, s0, :], xT_r[:, s0, :])
            nc.scalar.dma_start(xT_sb[:, s1, :], xT_r[:, s1, :])

            # zero g2 once (gpsimd — off critical path)
            nc.gpsimd.memset(g2_sb[:], 0.0)

            # warm up the PE p-state while the first xN DMA is in flight
            nc.vector.memset(scr_sb[:], 0.0)
            wu = psw.tile([128, 512], F32, tag="w", name="wu")
            for k in range(10):
                nc.tensor.matmul(wu[:, :512], scr_sb[:, 0:128],
                                 scr_sb[:, :512], start=(k == 0),
                                 stop=(k == 9))

            copy_engines = [nc.vector.tensor_copy, nc.scalar.copy]
            ce_idx = [0]

            def copy(dst, src):
                copy_engines[ce_idx[0] % 2](dst, src)
                ce_idx[0] += 1

            # ---- C = x.T x, upper triangle, 3 phases of 2 rows ----
            # row r covers cols r*128:768 (split into <=512 chunks)
            row_chunks = {0: [(0, 512), (512, 256)], 1: [(128, 512), (640, 128)],
                          2: [(256, 512)], 3: [(384, 384)],
                          4: [(512, 256)], 5: [(640, 128)]}

            def c_phase(rows):
                tiles = []
                for r in rows:
                    for c0, w in row_chunks[r]:
                        pt = psa.tile([128, 512], F32, tag="a", name=f"c{r}_{c0}")
                        tiles.append((r, c0, w, pt))
                for nt in range(NT):
                    for r, c0, w, pt in tiles:
                        nc.tensor.matmul(
                            pt[:, :w],
                            xn_t[nt][:, r * 128:(r + 1) * 128],
                            xn_t[nt][:, c0:c0 + w],
                            start=(nt == 0), stop=(nt == NT - 1),
                        )
                for r, c0, w, pt in tiles:
                    copy(c_sb[:, r, c0:c0 + w], pt[:, :w])

            def emit_mirror(i, j):
                # slot (j, i) := transpose of stored upper block (i, j)
                tp = psg.tile([128, 128], BF16, tag="g", name=f"tr{i}{j}")
                nc.tensor.transpose(
                    tp[:], c_sb[:, i, j * 128:(j + 1) * 128], id_sb[:]
                )
                nc.vector.tensor_copy(
                    c_sb[:, j, i * 128:(i + 1) * 128].bitcast(U32),
                    tp[:].bitcast(U32))

            def t1_stage(i):
                ua = psa.tile([128, 512], F32, tag="a", name=f"t1a{i}")
                ub = psa.tile([128, 512], F32, tag="a", name=f"t1b{i}")
                for j in range(DT):
                    lhsT = c_sb[:, j, i * 128:(i + 1) * 128]
                    nc.tensor.matmul(ua[:, :512], lhsT, wvT_sb[:, j, 0:512],
                                     start=(j == 0), stop=(j == DT - 1))
                    nc.tensor.matmul(ub[:, :256], lhsT, wvT_sb[:, j, 512:768],
                                     start=(j == 0), stop=(j == DT - 1))
                copy(t1_sb[:, i, 0:512], ua[:, :512])
                copy(t1_sb[:, i, 512:768], ub[:, :256])

            c_phase([0, 1])
            c_phase([2, 3])
            for i, j in [(0, 1), (0, 2), (0, 3), (0, 4), (0, 5),
                         (1, 2), (1, 3), (1, 4), (1, 5)]:
                emit_mirror(i, j)
            # T1 i=0,1 depend only on rows 0/1 blocks + batch-1 mirrors —
            # run them before C phase 4/5 to cover the mirror-copy latency
            t1_stage(0)
            c_phase([4, 5])
            t1_stage(1)
            for i, j in [(2, 3), (2, 4), (2, 5), (3, 4), (3, 5), (4, 5)]:
                emit_mirror(i, j)
            for i in range(2, DT):
                t1_stage(i)

            # ---- G (block-diag per pair) + M1T + M, software-pipelined ----
            def g_stage(t):
                gp = psg.tile([128, 128], F32, tag="g", name=f"g{t}")
                for j in range(DT):
                    nc.tensor.matmul(
                        gp[:],
                        wkT_sb[:, j, t * 128:(t + 1) * 128],
                        t1_sb[:, j, t * 128:(t + 1) * 128],
                        start=(j == 0), stop=(j == DT - 1),
                    )
                # tiny copies on vector only: M1T's dependency lands fast,
                # never queued behind the big m1t/m copies on scalar
                nc.vector.tensor_copy(g2_sb[0:64, t, 0:64], gp[0:64, 0:64])
                nc.vector.tensor_copy(g2_sb[64:128, t, 64:128],
                                      gp[64:128, 64:128])

            def m1t_stage(t):
                pa = psw.tile([128, 512], F32, tag="w", name=f"m1a{t}")
                pb = psw.tile([128, 512], F32, tag="w", name=f"m1b{t}")
                nc.tensor.matmul(pa[:, :512], g2_sb[:, t, :],
                                 wq_sb[:, t, 0:512], start=True, stop=True)
                nc.tensor.matmul(pb[:, :256], g2_sb[:, t, :],
                                 wq_sb[:, t, 512:768], start=True, stop=True)
                nc.scalar.copy(m1t_sb[:, t, 0:512], pa[:, :512])
                nc.scalar.copy(m1t_sb[:, t, 512:768], pb[:, :256])

            # M accumulators for dblocks 0,1 live across the G/M1T pipeline
            mtiles01 = []
            for db in (0, 1):
                ma = psa.tile([128, 512], F32, tag="a", name=f"ma{db}")
                mb = psa.tile([128, 512], F32, tag="a", name=f"mb{db}")
                mtiles01.append((db, ma, mb))

            def m_step(dbtiles, t):
                for db, ma, mb in dbtiles:
                    lhsT = m1t_sb[:, t, db * 128:(db + 1) * 128]
                    nc.tensor.matmul(ma[:, :512], lhsT, wfcT_sb[:, t, 0:512],
                                     start=(t == 0), stop=(t == DT - 1))
                    nc.tensor.matmul(mb[:, :256], lhsT, wfcT_sb[:, t, 512:768],
                                     start=(t == 0), stop=(t == DT - 1))

            g_stage(0)
            g_stage(1)
            m1t_stage(0)
            g_stage(2)
            m1t_stage(1)
            m_step(mtiles01, 0)
            g_stage(3)
            m1t_stage(2)
            m_step(mtiles01, 1)
            g_stage(4)
            m1t_stage(3)
            m_step(mtiles01, 2)
            g_stage(5)
            m1t_stage(4)
            m_step(mtiles01, 3)
            m1t_stage(5)
            m_step(mtiles01, 4)
            m_step(mtiles01, 5)
            for db, ma, mb in mtiles01:
                copy(m_sb[:, db, 0:512], ma[:, :512])
                copy(m_sb[:, db, 512:768], mb[:, :256])
            for phase in (1, 2):
                mts = []
                for db in (phase * 2, phase * 2 + 1):
                    ma = psa.tile([128, 512], F32, tag="a", name=f"ma{db}")
                    mb = psa.tile([128, 512], F32, tag="a", name=f"mb{db}")
                    mts.append((db, ma, mb))
                for t in range(DT):
                    m_step(mts, t)
                for db, ma, mb in mts:
                    copy(m_sb[:, db, 0:512], ma[:, :512])
                    copy(m_sb[:, db, 512:768], mb[:, :256])

            # ---- outT = M.T @ xT + bias ----
            for et in range(DT):
                oa = psa.tile([128, 512], F32, tag="a", name=f"oa{et}")
                ob = psa.tile([128, 512], F32, tag="a", name=f"ob{et}")
                ot = outsp.tile([128, N], BF16, tag="ot", name=f"ot{et}")
                if et < DT - 1:
                    for dt in range(DT):
                        lhsT = m_sb[:, dt, et * 128:(et + 1) * 128]
                        nc.tensor.matmul(oa[:, :512], lhsT,
                                         xT_sb[:, dt, 0:512],
                                         start=(dt == 0), stop=(dt == DT - 1))
                        nc.tensor.matmul(ob[:, :512], lhsT,
                                         xT_sb[:, dt, 512:1024],
                                         start=(dt == 0), stop=(dt == DT - 1))
                    nc.scalar.add(ot[:, 0:512], oa[:, :512],
                                  bias_sb[:, et:et + 1])
                    nc.vector.tensor_scalar_add(ot[:, 512:1024], ob[:, :512],
                                                bias_sb[:, et:et + 1])
                    nc.sync.dma_start(outT_r[:, et, :], ot[:])
                else:
                    # last tile: finish the low n-half first so its bias+DMA
                    # overlaps the high half's matmuls, shortening the tail
                    for dt in range(DT):
                        nc.tensor.matmul(oa[:, :512],
                                         m_sb[:, dt, et * 128:(et + 1) * 128],
                                         xT_sb[:, dt, 0:512],
                                         start=(dt == 0), stop=(dt == DT - 1))
                    nc.scalar.add(ot[:, 0:256], oa[:, 0:256],
                                  bias_sb[:, et:et + 1])
                    nc.sync.dma_start(outT_r[:, et, 0:256], ot[:, 0:256])
                    nc.vector.tensor_scalar_add(ot[:, 256:512], oa[:, 256:512],
                                                bias_sb[:, et:et + 1])
                    nc.scalar.dma_start(outT_r[:, et, 256:512], ot[:, 256:512])
                    for dt in range(DT):
                        nc.tensor.matmul(ob[:, :512],
                                         m_sb[:, dt, et * 128:(et + 1) * 128],
                                         xT_sb[:, dt, 512:1024],
                                         start=(dt == 0), stop=(dt == DT - 1))
                    nc.scalar.add(ot[:, 512:768], ob[:, 0:256],
                                  bias_sb[:, et:et + 1])
                    nc.sync.dma_start(outT_r[:, et, 512:768], ot[:, 512:768])
                    nc.vector.tensor_scalar_add(ot[:, 768:1024],
                                                ob[:, 256:512],
                                                bias_sb[:, et:et + 1])
                    nc.scalar.dma_start(outT_r[:, et, 768:1024],
                                        ot[:, 768:1024])

            if debug_dumps:
                for nm, sb in [("d_c", c_sb), ("d_t1", t1_sb),
                               ("d_g2", g2_sb), ("d_m1t", m1t_sb),
                               ("d_m", m_sb)]:
                    nc.sync.dma_start(dbg[nm], sb)

    nc.compile()
    return nc


_NC_CACHE = None
LAST_EXEC_NS = None
LAST_RES = None


def kernel(x, w_qkv, w_fc, b_fc, _trace=False):
    global _NC_CACHE, LAST_EXEC_NS, LAST_RES
    x = np.asarray(x, dtype=np.float32)
    w_qkv = np.asarray(w_qkv, dtype=np.float32)
    w_fc = np.asarray(w_fc, dtype=np.float32)
    b_fc = np.asarray(b_fc, dtype=np.float32)

    if _NC_CACHE is None:
        _NC_CACHE = _build_program()
    nc = _NC_CACHE

    bf = ml_dtypes.bfloat16
    wvT = np.ascontiguousarray(w_qkv[2 * D:].T).astype(bf)
    wkT = np.ascontiguousarray(w_qkv[D:2 * D].T).astype(bf)
    wq = np.ascontiguousarray(SCALE * w_qkv[:D]).astype(bf)
    wfcT = np.ascontiguousarray(w_fc.T).astype(bf)
    ident = np.eye(128, dtype=bf)

    in_maps = []
    for b in range(B):
        in_maps.append({
            "xN": x[b].astype(bf),
            "xT": np.ascontiguousarray(x[b].T).astype(bf),
            "wvT": wvT, "wkT": wkT, "wq": wq, "wfcT": wfcT,
            "bfc": b_fc, "ident": ident,
        })

    res = bass_utils.run_bass_kernel_spmd(
        nc, in_maps, core_ids=list(range(B)), trace=_trace
    )
    LAST_EXEC_NS = res.exec_time_ns
    LAST_RES = res
    out = np.stack([res.results[b]["outT"].astype(np.float32).T
                    for b in range(B)])
    return np.ascontiguousarray(out)


# revision 28
# speedup vs baseline: 1.0039x; 1.0010x over previous
"""Trainium2 Bass kernel for nn_Attention_84585085927925 — bf16 M-folded Gram.

Reference (per batch element b, all fp32):
    qkv = x @ w_qkv.T ; q,k,v heads of 64 ; attn = sqrt(64) * q @ k.T (NO
    softmax) ; out = attn @ v ; out = out @ w_fc.T + b_fc

No softmax => attention is linear; fold k/v AND the q/fc projections into a
single per-batch effective matrix M:
    out = x @ M + b_fc,   M = s * sum_h wq_h.T G_h wfc[:,h].T,
    G_h = wk_h C wv_h.T,  C = x.T x  (symmetric)
Per-core pipeline (one batch element per NeuronCore, 8 cores):
    C    upper-triangle blocks via PSUM, mirrored by PE transpose
    T1   = C @ wv.T                       [768,768]
    G    = wk_pair @ T1[:,pair]  (block-diag per head pair)
    M1T  = G_blkdiag.T @ (s*wq)_pair      [128,768] per pair
    M    = M1T.T @ wfc.T                  [768,768]
    outT = M.T @ xT + b_fc                [768,1024]
All matmuls bf16 (1 cyc/row, FWL weight loads); fp32 PSUM accumulate.
"""

import numpy as np
import ml_dtypes

import concourse.bass as bass  # noqa: F401  (registers engine namespaces)
import concourse.mybir as mybir
import concourse.tile as tile
from concourse import bacc, bass_utils

F32 = mybir.dt.float32
BF16 = mybir.dt.bfloat16
U32 = mybir.dt.uint32

B, N, D, H = 8, 1024, 768, 12
HD = D // H            # 64
SCALE = float(np.sqrt(HD))
DT = D // 128           # 6 blocks of 128 along feature dims
NT = N // 128           # 8 token tiles


def _build_program(debug_dumps=False, num_devices=B):
    nc = bacc.Bacc(
        trn_type="TRN2", target_bir_lowering=False, debug=False,
        num_devices=num_devices
    )
    xN_d = nc.dram_tensor("xN", [N, D], BF16, kind="ExternalInput").ap()
    xT_d = nc.dram_tensor("xT", [D, N], BF16, kind="ExternalInput").ap()
    wvT_d = nc.dram_tensor("wvT", [D, D], BF16, kind="ExternalInput").ap()
    wkT_d = nc.dram_tensor("wkT", [D, D], BF16, kind="ExternalInput").ap()
    wq_d = nc.dram_tensor("wq", [D, D], BF16, kind="ExternalInput").ap()
    wfcT_d = nc.dram_tensor("wfcT", [D, D], BF16, kind="ExternalInput").ap()
    bfc_d = nc.dram_tensor("bfc", [D], F32, kind="ExternalInput").ap()
    id_d = nc.dram_tensor("ident", [128, 128], BF16, kind="ExternalInput").ap()
    outT_d = nc.dram_tensor("outT", [D, N], BF16, kind="ExternalOutput").ap()
    dbg = {}
    if debug_dumps:
        for nm, shape in [("d_c", [128, DT, D]), ("d_t1", [128, DT, D]),
                          ("d_g2", [128, DT, 128]), ("d_m1t", [128, DT, D]),
                          ("d_m", [128, DT, D])]:
            dbg[nm] = nc.dram_tensor(nm, shape, BF16,
                                     kind="ExternalOutput").ap()

    with tile.TileContext(nc) as tc:
        with tc.tile_pool(name="big", bufs=1) as big, \
             tc.tile_pool(name="outsp", bufs=3) as outsp, \
             tc.tile_pool(name="psa", bufs=4, space="PSUM") as psa, \
             tc.tile_pool(name="psw", bufs=2, space="PSUM") as psw, \
             tc.tile_pool(name="psg", bufs=2, space="PSUM") as psg:

        # PSUM static budget: psa 4 banks + psw 2 + psg 2 = 8.

            xn_t = [big.tile([128, D], BF16, name=f"xn{o}") for o in range(NT)]
            xT_sb = big.tile([128, DT, N], BF16, name="xT_sb")
            wvT_sb = big.tile([128, DT, D], BF16, name="wvT_sb")
            wkT_sb = big.tile([128, DT, D], BF16, name="wkT_sb")
            wq_sb = big.tile([128, DT, D], BF16, name="wq_sb")
            wfcT_sb = big.tile([128, DT, D], BF16, name="wfcT_sb")
            c_sb = big.tile([128, DT, D], BF16, name="c_sb")
            t1_sb = big.tile([128, DT, D], BF16, name="t1_sb")
            g2_sb = big.tile([128, DT, 128], BF16, name="g2_sb")
            m1t_sb = big.tile([128, DT, D], BF16, name="m1t_sb")
            m_sb = big.tile([128, DT, D], BF16, name="m_sb")
            bias_sb = big.tile([128, DT], F32, name="bias_sb")
            id_sb = big.tile([128, 128], BF16, name="id_sb")
            scr_sb = big.tile([128, 512], BF16, name="scr_sb")

            xN_r = xN_d.rearrange("(o p) e -> p o e", p=128)
            xT_r = xT_d.rearrange("(o p) n -> p o n", p=128)
            wvT_r = wvT_d.rearrange("(o p) e -> p o e", p=128)
            wkT_r = wkT_d.rearrange("(o p) e -> p o e", p=128)
            wq_r = wq_d.rearrange("(o p) e -> p o e", p=128)
            wfcT_r = wfcT_d.rearrange("(o p) e -> p o e", p=128)
            outT_r = outT_d.rearrange("(o p) n -> p o n", p=128)

            # ---- DMA in on two hwdge queues (sync + scalar) ----
            # sync queue: xN even tiles first (C starts immediately)
            # scalar queue: xN odd tiles, then remaining weights
            nc.sync.dma_start(xn_t[0][:], xN_r[:, 0, :])
            nc.scalar.dma_start(xn_t[1][:], xN_r[:, 1, :])
            nc.sync.dma_start(xn_t[2][:], xN_r[:, 2, :])
            nc.scalar.dma_start(xn_t[3][:], xN_r[:, 3, :])
            nc.sync.dma_start(xn_t[4][:], xN_r[:, 4, :])
            nc.scalar.dma_start(xn_t[5][:], xN_r[:, 5, :])
            nc.sync.dma_start(xn_t[6][:], xN_r[:, 6, :])
            nc.scalar.dma_start(xn_t[7][:], xN_r[:, 7, :])
            nc.sync.dma_start(id_sb[:], id_d)
            nc.sync.dma_start(bias_sb[:], bfc_d.rearrange("(o p) -> p o", p=128))
            # strict first-needed priority, halves split across both queues
            s0, s1 = slice(0, 3), slice(3, 6)
            nc.sync.dma_start(wvT_sb[:, s0, :], wvT_r[:, s0, :])
            nc.scalar.dma_start(wvT_sb[:, s1, :], wvT_r[:, s1, :])
            nc.sync.dma_start(wkT_sb[:, s0, :], wkT_r[:, s0, :])
            nc.scalar.dma_start(wkT_sb[:, s1, :], wkT_r[:, s1, :])
            nc.sync.dma_start(wq_sb[:, s0, :], wq_r[:, s0, :])
            nc.scalar.dma_start(wq_sb[:, s1, :], wq_r[:, s1, :])
            nc.sync.dma_start(wfcT_sb[:, s0, :], wfcT_r[:, s0, :])
            nc.scalar.dma_start(wfcT_sb[:, s1, :], wfcT_r[:, s1, :])
            nc.sync.dma_start(xT_sb[:, s0, :], xT_r[:, s0, :])
            nc.scalar.dma_start(xT_sb[:, s1, :], xT_r[:, s1, :])

            # zero g2 once (gpsimd — off critical path)
            nc.gpsimd.memset(g2_sb[:], 0.0)

            # warm up the PE p-state while the first xN DMA is in flight
            nc.vector.memset(scr_sb[:], 0.0)
            wu = psw.tile([128, 512], F32, tag="w", name="wu")
            for k in range(10):
                nc.tensor.matmul(wu[:, :512], scr_sb[:, 0:128],
                                 scr_sb[:, :512], start=(k == 0),
                                 stop=(k == 9))

            copy_engines = [nc.vector.tensor_copy, nc.scalar.copy]
            ce_idx = [0]

            def copy(dst, src):
                copy_engines[ce_idx[0] % 2](dst, src)
                ce_idx[0] += 1

            # ---- C = x.T x, upper triangle, 3 phases of 2 rows ----
            # row r covers cols r*128:768 (split into <=512 chunks)
            row_chunks = {0: [(0, 512), (512, 256)], 1: [(128, 512), (640, 128)],
                          2: [(256, 512)], 3: [(384, 384)],
                          4: [(512, 256)], 5: [(640, 128)]}

            def c_phase(rows):
                tiles = []
                for r in rows:
                    for c0, w in row_chunks[r]:
                        pt = psa.tile([128, 512], F32, tag="a", name=f"c{r}_{c0}")
                        tiles.append((r, c0, w, pt))
                for nt in range(NT):
                    for r, c0, w, pt in tiles:
                        nc.tensor.matmul(
                            pt[:, :w],
                            xn_t[nt][:, r * 128:(r + 1) * 128],
                            xn_t[nt][:, c0:c0 + w],
                            start=(nt == 0), stop=(nt == NT - 1),
                        )
                for r, c0, w, pt in tiles:
                    copy(c_sb[:, r, c0:c0 + w], pt[:, :w])

            def emit_mirror(i, j):
                # slot (j, i) := transpose of stored upper block (i, j)
                tp = psg.tile([128, 128], BF16, tag="g", name=f"tr{i}{j}")
                nc.tensor.transpose(
                    tp[:], c_sb[:, i, j * 128:(j + 1) * 128], id_sb[:]
                )
                nc.vector.tensor_copy(
                    c_sb[:, j, i * 128:(i + 1) * 128].bitcast(U32),
                    tp[:].bitcast(U32))

            def t1_stage(i):
                ua = psa.tile([128, 512], F32, tag="a", name=f"t1a{i}")
                ub = psa.tile([128, 512], F32, tag="a", name=f"t1b{i}")
                for j in range(DT):
                    lhsT = c_sb[:, j, i * 128:(i + 1) * 128]
                    nc.tensor.matmul(ua[:, :512], lhsT, wvT_sb[:, j, 0:512],
                                     start=(j == 0), stop=(j == DT - 1))
                    nc.tensor.matmul(ub[:, :256], lhsT, wvT_sb[:, j, 512:768],
                                     start=(j == 0), stop=(j == DT - 1))
                copy(t1_sb[:, i, 0:512], ua[:, :512])
                copy(t1_sb[:, i, 512:768], ub[:, :256])

            c_phase([0, 1])
            c_phase([2, 3])
            for i, j in [(0, 1), (0, 2), (0, 3), (0, 4), (0, 5),
                         (1, 2), (1, 3), (1, 4), (1, 5)]:
                emit_mirror(i, j)
            # T1 i=0,1 depend only on rows 0/1 blocks + batch-1 mirrors —
            # run them before C phase 4/5 to cover the mirror-copy latency
            t1_stage(0)
            c_phase([4, 5])
            t1_stage(1)
            for i, j in [(2, 3), (2, 4), (2, 5), (3, 4), (3, 5), (4, 5)]:
                emit_mirror(i, j)
            for i in range(2, DT):
                t1_stage(i)

            # ---- G (block-diag per pair) + M1T + M, software-pipelined ----
            def g_stage(t):
                gp = psg.tile([128, 128], F32, tag="g", name=f"g{t}")
                for j in range(DT):
                    nc.tensor.matmul(
                        gp[:],
                        wkT_sb[:, j, t * 128:(t + 1) * 128],
                        t1_sb[:, j, t * 128:(t + 1) * 128],
                        start=(j == 0), stop=(j == DT - 1),
                    )
                # tiny copies on vector only: M1T's dependency lands fast,
                # never queued behind the big m1t/m copies on scalar
                nc.vector.tensor_copy(g2_sb[0:64, t, 0:64], gp[0:64, 0:64])
                nc.vector.tensor_copy(g2_sb[64:128, t, 64:128],
                                      gp[64:128, 64:128])

            def m1t_stage(t):
                pa = psw.tile([128, 512], F32, tag="w", name=f"m1a{t}")
                pb = psw.tile([128, 512], F32, tag="w", name=f"m1b{t}")
                nc.tensor.matmul(pa[:, :512], g2_sb[:, t, :],
                                 wq_sb[:, t, 0:512], start=True, stop=True)
                nc.tensor.matmul(pb[:, :256], g2_sb[:, t, :],
                                 wq_sb[:, t, 512:768], start=True, stop=True)
                nc.scalar.copy(m1t_sb[:, t, 0:512], pa[:, :512])
                nc.scalar.copy(m1t_sb[:, t, 512:768], pb[:, :256])

            # M accumulators for dblocks 0,1 live across the G/M1T pipeline
            mtiles01 = []
            for db in (0, 1):
                ma = psa.tile([128, 512], F32, tag="a", name=f"ma{db}")
                mb = psa.tile([128, 512], F32, tag="a", name=f"mb{db}")
                mtiles01.append((db, ma, mb))

            def m_step(dbtiles, t):
                for db, ma, mb in dbtiles:
                    lhsT = m1t_sb[:, t, db * 128:(db + 1) * 128]
                    nc.tensor.matmul(ma[:, :512], lhsT, wfcT_sb[:, t, 0:512],
                                     start=(t == 0), stop=(t == DT - 1))
                    nc.tensor.matmul(mb[:, :256], lhsT, wfcT_sb[:, t, 512:768],
                                     start=(t == 0), stop=(t == DT - 1))

            g_stage(0)
            g_stage(1)
            m1t_stage(0)
            g_stage(2)
            m1t_stage(1)
            m_step(mtiles01, 0)
            g_stage(3)
            m1t_stage(2)
            m_step(mtiles01, 1)
            g_stage(4)
            m1t_stage(3)
            m_step(mtiles01, 2)
            g_stage(5)
            m1t_stage(4)
            m_step(mtiles01, 3)
            m1t_stage(5)
            m_step(mtiles01, 4)
            m_step(mtiles01, 5)
            for db, ma, mb in mtiles01:
                copy(m_sb[:, db, 0:512], ma[:, :512])
                copy(m_sb[:, db, 512:768], mb[:, :256])
            for phase in (1, 2):
                mts = []
                for db in (phase * 2, phase * 2 + 1):
                    ma = psa.tile([128, 512], F32, tag="a", name=f"ma{db}")
                    mb = psa.tile([128, 512], F32, tag="a", name=f"mb{db}")
                    mts.append((db, ma, mb))
                for t in range(DT):
                    m_step(mts, t)
                for db, ma, mb in mts:
                    copy(m_sb[:, db, 0:512], ma[:, :512])
                    copy(m_sb[:, db, 512:768], mb[:, :256])

            # ---- outT = M.T @ xT + bias ----
            for et in range(DT):
                oa = psa.tile([128, 512], F32, tag="a", name=f"oa{et}")
                ob = psa.tile([128, 512], F32, tag="a", name=f"ob{et}")
                ot = outsp.tile([128, N], BF16, tag="ot", name=f"ot{et}")
                if et < DT - 1:
                    for dt in range(DT):
                        lhsT = m_sb[:, dt, et * 128:(et + 1) * 128]
                        nc.tensor.matmul(oa[:, :512], lhsT,
                                         xT_sb[:, dt, 0:512],
                                         start=(dt == 0), stop=(dt == DT - 1))
                        nc.tensor.matmul(ob[:, :512], lhsT,
                                         xT_sb[:, dt, 512:1024],
                                         start=(dt == 0), stop=(dt == DT - 1))
                    nc.scalar.add(ot[:, 0:512], oa[:, :512],
                                  bias_sb[:, et:et + 1])
                    nc.vector.tensor_scalar_add(ot[:, 512:1024], ob[:, :512],
                                                bias_sb[:, et:et + 1])
                    nc.sync.dma_start(outT_r[:, et, :], ot[:])
                else:
                    # last tile: finish the low n-half first so its bias+DMA
                    # overlaps the high half's matmuls, shortening the tail
                    for dt in range(DT):
                        nc.tensor.matmul(oa[:, :512],
                                         m_sb[:, dt, et * 128:(et + 1) * 128],
                                         xT_sb[:, dt, 0:512],
                                         start=(dt == 0), stop=(dt == DT - 1))
                    nc.scalar.add(ot[:, 0:256], oa[:, 0:256],
                                  bias_sb[:, et:et + 1])
                    nc.vector.tensor_scalar_add(ot[:, 256:512], oa[:, 256:512],
                                                bias_sb[:, et:et + 1])
                    nc.sync.dma_start(outT_r[:, et, 0:256], ot[:, 0:256])
                    nc.sync.dma_start(outT_r[:, et, 256:512], ot[:, 256:512])
                    for dt in range(DT):
                        nc.tensor.matmul(ob[:, :512],
                                         m_sb[:, dt, et * 128:(et + 1) * 128],
                                         xT_sb[:, dt, 512:1024],
                                         start=(dt == 0), stop=(dt == DT - 1))
                    # both final bias chunks run concurrently; all DMA issues
                    # go to the idle sync queue so none waits behind a bias op
                    nc.scalar.add(ot[:, 512:768], ob[:, 0:256],
                                  bias_sb[:, et:et + 1])
                    nc.vector.tensor_scalar_add(ot[:, 768:1024],
                                                ob[:, 256:512],
                                                bias_sb[:, et:et + 1])
                    nc.sync.dma_start(outT_r[:, et, 512:768], ot[:, 512:768])
                    nc.sync.dma_start(outT_r[:, et, 768:1024],
                                      ot[:, 768:1024])

            if debug_dumps:
                for nm, sb in [("d_c", c_sb), ("d_t1", t1_sb),
                               ("d_g2", g2_sb), ("d_m1t", m1t_sb),
                               ("d_m", m_sb)]:
                    nc.sync.dma_start(dbg[nm], sb)

    nc.compile()
    return nc


_NC_CACHE = None
LAST_EXEC_NS = None
LAST_RES = None


def kernel(x, w_qkv, w_fc, b_fc, _trace=False):
    global _NC_CACHE, LAST_EXEC_NS, LAST_RES
    x = np.asarray(x, dtype=np.float32)
    w_qkv = np.asarray(w_qkv, dtype=np.float32)
    w_fc = np.asarray(w_fc, dtype=np.float32)
    b_fc = np.asarray(b_fc, dtype=np.float32)

    if _NC_CACHE is None:
        _NC_CACHE = _build_program()
    nc = _NC_CACHE

    bf = ml_dtypes.bfloat16
    wvT = np.ascontiguousarray(w_qkv[2 * D:].T).astype(bf)
    wkT = np.ascontiguousarray(w_qkv[D:2 * D].T).astype(bf)
    wq = np.ascontiguousarray(SCALE * w_qkv[:D]).astype(bf)
    wfcT = np.ascontiguousarray(w_fc.T).astype(bf)
    ident = np.eye(128, dtype=bf)

    in_maps = []
    for b in range(B):
        in_maps.append({
            "xN": x[b].astype(bf),
            "xT": np.ascontiguousarray(x[b].T).astype(bf),
            "wvT": wvT, "wkT": wkT, "wq": wq, "wfcT": wfcT,
            "bfc": b_fc, "ident": ident,
        })

    res = bass_utils.run_bass_kernel_spmd(
        nc, in_maps, core_ids=list(range(B)), trace=_trace
    )
    LAST_EXEC_NS = res.exec_time_ns
    LAST_RES = res
    out = np.stack([res.results[b]["outT"].astype(np.float32).T
                    for b in range(B)])
    return np.ascontiguousarray(out)
